# revision 1
# baseline (speedup 1.0000x reference)
"""DirectTuckerNet forward on 8 Trainium2 NeuronCores.

    out = A @ G @ kron(C, B).T        # (I, K*J), fp32
    A: (512, 4)  B: (512, 4)  C: (512, 4)  G: (4, 16)

Equivalent per-element form (M = A @ G reshaped to (I, R3, R2)):
    out[i, k*J + j] = sum_{p,q} M[i, p*4+q] * C[k, p] * B[j, q]

Sharding: rows of C (the K dim) across the 8 cores.  Each core gets a
(KS=64)-row slice of C and produces the contiguous (512, KS*J) = 64 MiB
column block of the output, so the kernel is HBM-write bound
(~358 GB/s/core -> ~190 us).

Per-core device program:
  MT (16, 512) = G.T @ A.T                       one PE matmul
  for each k in the 64-slice:
      rhs_k (16, 512) = tile(B.T,(4,1)) * C16[:, k]   per-partition scaled copy
      for each 128-row block m of I:
          psum (128, 512) = MT[:, m-block].T @ rhs_k  PE matmul (K=16)
          copy psum -> staging SBUF tile              DVE/ACT alternating
  DMA staging tiles out in 4 MiB chunks.

Host only does layout prep of the tiny operands (transpose/replicate,
no arithmetic) and the final concat of the 8 column blocks.
"""

import numpy as np

import concourse.bass as bass
import concourse.mybir as mybir
from concourse import bacc, tile
from concourse.bass_utils import run_bass_kernel_spmd

_f32 = mybir.dt.float32
_f32r = mybir.dt.float32r
_bf16 = mybir.dt.bfloat16

I, J, K = 512, 512, 512
R = 4                      # R1 = R2 = R3
RR = R * R                 # 16, the contraction dim
NCORES = 8
KS = K // NCORES           # 64 k-rows per core
KCHUNK = 16                # k-values per staged output chunk
NCHUNK = KS // KCHUNK      # 4
MBLK = I // 128            # 4 row blocks of I

# 'f32'   : plain fp32 matmuls (4 cyc/row on PE, bit-accurate)
# 'f32r'  : TF32-like fast path (1 cyc/row, ~1e-4 rel err)
# 'bf16x3': 3-product hi/lo bf16 split (1 cyc/row x3, ~1e-6 rel err)
MATMUL_MODE = "f32"


def _build_nc(mode: str = MATMUL_MODE):
    nc = bacc.Bacc()
    at = nc.dram_tensor("at", [R, I], _f32, kind="ExternalInput")        # A.T
    g = nc.dram_tensor("g", [R, RR], _f32, kind="ExternalInput")         # G
    bt16 = nc.dram_tensor("bt16", [RR, J], _f32, kind="ExternalInput")   # tile(B.T, (4,1))
    ct16 = nc.dram_tensor("ct16", [RR, KS], _f32, kind="ExternalInput")  # repeat(Cs.T, 4, axis=0)
    o = nc.dram_tensor("o", [I, KS * J], _f32, kind="ExternalOutput")

    mm_dt = {"f32": _f32, "f32r": _f32r, "bf16x3": _bf16}[mode]

    with tile.TileContext(nc) as tc:
        with (
            tc.tile_pool(name="singles", bufs=1) as singles,
            tc.tile_pool(name="ps_mt", bufs=1, space="PSUM") as ps_mt,
            tc.tile_pool(name="ps", bufs=6, space="PSUM") as ps,
            tc.tile_pool(name="rh", bufs=2) as rh_pool,
            tc.tile_pool(name="ot", bufs=2) as ot_pool,
        ):
            at_s = singles.tile([R, I], _f32)
            g_s = singles.tile([R, RR], _f32)
            bt_s = singles.tile([RR, J], _f32)
            ct_s = singles.tile([RR, KS], _f32)
            nc.sync.dma_start(out=at_s, in_=at[:, :])
            nc.sync.dma_start(out=g_s, in_=g[:, :])
            nc.sync.dma_start(out=bt_s, in_=bt16[:, :])
            nc.sync.dma_start(out=ct_s, in_=ct16[:, :])

            # MT[t, i] = sum_r G[r, t] * A.T[r, i]   -> (16, 512)
            mt_ps = ps_mt.tile([RR, I], _f32)
            nc.tensor.matmul(mt_ps, g_s, at_s, start=True, stop=True)

            if mode == "bf16x3":
                mt_hi = singles.tile([RR, I], _bf16, tag="mt_hi")
                mt_rest = singles.tile([RR, I], _f32, tag="mt_rest")
                mt_lo = singles.tile([RR, I], _bf16, tag="mt_lo")
                nc.vector.tensor_copy(mt_hi, mt_ps)
                nc.vector.tensor_sub(mt_rest, mt_ps, mt_hi)
                nc.vector.tensor_copy(mt_lo, mt_rest)
            else:
                mt_s = singles.tile([RR, I], mm_dt, tag="mt")
                nc.vector.tensor_copy(mt_s, mt_ps)

            for c in range(NCHUNK):
                if mode == "bf16x3":
                    rh_f = rh_pool.tile([RR, KCHUNK * J], _f32, tag="rh_f")
                    rh_hi = rh_pool.tile([RR, KCHUNK * J], _bf16, tag="rh_hi")
                    rh_rest = rh_pool.tile([RR, KCHUNK * J], _f32, tag="rh_rest")
                    rh_lo = rh_pool.tile([RR, KCHUNK * J], _bf16, tag="rh_lo")
                else:
                    rh = rh_pool.tile([RR, KCHUNK * J], mm_dt, tag="rh")
                for kl in range(KCHUNK):
                    k = c * KCHUNK + kl
                    js = slice(kl * J, (kl + 1) * J)
                    if mode == "bf16x3":
                        nc.scalar.mul(rh_f[:, js], bt_s, ct_s[:, k : k + 1])
                        nc.vector.tensor_copy(rh_hi[:, js], rh_f[:, js])
                        nc.vector.tensor_sub(rh_rest[:, js], rh_f[:, js], rh_hi[:, js])
                        nc.vector.tensor_copy(rh_lo[:, js], rh_rest[:, js])
                    elif kl % 2 == 0:
                        nc.scalar.mul(rh[:, js], bt_s, ct_s[:, k : k + 1])
                    else:
                        nc.vector.tensor_scalar_mul(rh[:, js], bt_s, ct_s[:, k : k + 1])

                for m in range(MBLK):
                    ms = slice(m * 128, (m + 1) * 128)
                    ot = ot_pool.tile([128, KCHUNK * J], _f32, tag="ot")
                    for kl in range(KCHUNK):
                        js = slice(kl * J, (kl + 1) * J)
                        pt = ps.tile([128, J], _f32, tag="pt")
                        if mode == "bf16x3":
                            nc.tensor.matmul(
                                pt, mt_hi[:, ms], rh_hi[:, js], start=True, stop=False
                            )
                            nc.tensor.matmul(
                                pt, mt_hi[:, ms], rh_lo[:, js], start=False, stop=False
                            )
                            nc.tensor.matmul(
                                pt, mt_lo[:, ms], rh_hi[:, js], start=False, stop=True
                            )
                        else:
                            nc.tensor.matmul(
                                pt, mt_s[:, ms], rh[:, js], start=True, stop=True
                            )
                        # alternate the PSUM->SBUF copy between DVE and ACT
                        if kl % 2 == 0:
                            nc.vector.tensor_copy(ot[:, js], pt)
                        else:
                            nc.scalar.copy(ot[:, js], pt)
                    nc.sync.dma_start(
                        out=o[ms, c * KCHUNK * J : (c + 1) * KCHUNK * J], in_=ot
                    )

    nc.finalize()
    return nc


_NC_CACHE: dict[str, object] = {}


def kernel(A: np.ndarray, B: np.ndarray, C: np.ndarray, G: np.ndarray) -> np.ndarray:
    A = np.asarray(A, dtype=np.float32)
    B = np.asarray(B, dtype=np.float32)
    C = np.asarray(C, dtype=np.float32)
    G = np.asarray(G, dtype=np.float32)

    at = np.ascontiguousarray(A.T)                      # (4, 512)
    bt16 = np.ascontiguousarray(np.tile(B.T, (R, 1)))   # (16, 512), row t -> B[:, t%4]
    g = np.ascontiguousarray(G)                         # (4, 16)

    if MATMUL_MODE not in _NC_CACHE:
        _NC_CACHE[MATMUL_MODE] = _build_nc(MATMUL_MODE)
    nc = _NC_CACHE[MATMUL_MODE]

    in_maps = []
    for d in range(NCORES):
        cs = C[d * KS : (d + 1) * KS]                   # (64, 4)
        ct16 = np.ascontiguousarray(np.repeat(cs.T, R, axis=0))  # (16, 64), row t -> C[:, t//4]
        in_maps.append({"at": at, "g": g, "bt16": bt16, "ct16": ct16})

    res = run_bass_kernel_spmd(nc, in_maps, list(range(NCORES)))
    return np.concatenate([res.results[d]["o"] for d in range(NCORES)], axis=1)


# revision 13
# speedup vs baseline: 57.5657x; 57.5657x over previous
"""DirectTuckerNet forward on 8 Trainium2 NeuronCores.

    out = A @ G @ kron(C, B).T        # (I, K*J), fp32
    A: (512, 4)  B: (512, 4)  C: (512, 4)  G: (4, 16)

Equivalent per-element form (M = A @ G reshaped to (I, R3, R2)):
    out[i, k*J + j] = sum_{p,q} M[i, p*4+q] * C[k, p] * B[j, q]

Sharding: rows of C (the K dim) across the 8 cores.  Each core gets a
(KS=64)-row slice of C and produces the contiguous (512, KS*J) = 64 MiB
column block of the output, so the kernel is HBM-write bound.

The contraction (over the 16 (p,q) pairs) runs on the PE array.  Plain
fp32 matmul costs 4 cycles/row; the f32r (TF32-like, 12-bit mantissa)
path costs 1 cycle/row but loses ~1e-4.  Mode 'f32rk48' (default) gets
fp32-grade accuracy at f32r speed with a hi/lo split fused into ONE
K=48 matmul per output tile:

    x = xh + xl,  xh = f32r(x),  xl = f32r(x - xh)   (exact to ~2^-24)
    M @ R = Mh@Rh + Ml@Rh + Mh@Rl  (+ Ml@Rl ~ 2^-24, dropped)

  stationary MT3 (48, 512) = [Mh; Ml; Mh]     built once
  moving    RH3 (48, 512)  = [Rh; Rh; Rl]     per k:
      one 48-partition ACT scaled-copy writes Rh everywhere (f32r round),
      GPSIMD computes the unrounded product on rows 32-47,
      DVE overwrites rows 32-47 with Rl = f32r(b*c - Rh)
      (engine partition bases must be 32-aligned, hence this layout).

  psum (128,512) = MT3[:,m].T @ RH3[:,js]     one matmul per (m, k)
  PSUM -> SBUF copies alternate DVE/ACT; DMA out in 4 MiB chunks.

Host only does layout prep of the tiny operands (transpose/replicate,
no arithmetic) and the final concat of the 8 column blocks.

Modes: 'f32'     plain fp32 matmuls (slowest, exact)
       'f32r'    TF32-like single product (~1e-4 rel err)
       'f32rk48' fused hi/lo split, one K=48 matmul (fast, ~3e-7)
"""

import numpy as np

import concourse.bass as bass
import concourse.mybir as mybir
from concourse import bacc, tile
from concourse.bass_utils import run_bass_kernel_spmd

_f32 = mybir.dt.float32
_f32r = mybir.dt.float32r

I, J, K = 512, 512, 512
R = 4                      # R1 = R2 = R3
RR = R * R                 # 16, the contraction dim
RR3 = 3 * RR               # 48, the stacked contraction dim
NCORES = 8
KS = K // NCORES           # 64 k-rows per core
KCHUNK = 16                # k-values per staged output chunk
NCHUNK = KS // KCHUNK      # 4
MBLK = I // 128            # 4 row blocks of I

MATMUL_MODE = "f32rk48"


def _build_nc(
    mode: str = MATMUL_MODE,
    repeat: int = 1,
    *,
    sub_engine: str = "gpsimd",   # engine for the Rl subtraction
    ps_bufs: int = 7,
    ot_bufs: int = 2,
    copy_mod: int = 4,            # 1/copy_mod of PSUM copies go to DVE
):
    """repeat>1 re-runs the whole compute body (idempotent writes) so HW
    device time can be measured as a slope across repeat counts."""
    nc = bacc.Bacc()
    at = nc.dram_tensor("at", [R, I], _f32, kind="ExternalInput")       # A.T
    g = nc.dram_tensor("g", [R, RR], _f32, kind="ExternalInput")        # G
    # B.T replicated: row t of bt is B[:, t % 4] (3x16 rows).
    bt = nc.dram_tensor("bt", [RR3, J], _f32, kind="ExternalInput")
    # C-slice transposed+replicated: row t of ct is C[:, (t % 16) // 4].
    ct = nc.dram_tensor("ct", [RR3, KS], _f32, kind="ExternalInput")
    o = nc.dram_tensor("o", [I, KS * J], _f32, kind="ExternalOutput")

    mm_dt = _f32 if mode == "f32" else _f32r

    with tile.TileContext(nc) as tc:
        with (
            tc.tile_pool(name="singles", bufs=1) as singles,
            tc.tile_pool(name="ps_mt", bufs=1, space="PSUM") as ps_mt,
            tc.tile_pool(name="ps", bufs=ps_bufs, space="PSUM") as ps,
            tc.tile_pool(name="rh", bufs=2) as rh_pool,
            tc.tile_pool(name="rf", bufs=3) as rf_pool,
            tc.tile_pool(name="ot", bufs=ot_bufs) as ot_pool,
        ):
            at_s = singles.tile([R, I], _f32)
            g_s = singles.tile([R, RR], _f32)
            bt_s = singles.tile([RR3, J], _f32)
            ct_s = singles.tile([RR3, KS], _f32)
            nc.sync.dma_start(out=at_s, in_=at[:, :])
            nc.sync.dma_start(out=g_s, in_=g[:, :])
            nc.sync.dma_start(out=bt_s, in_=bt[:, :])
            nc.sync.dma_start(out=ct_s, in_=ct[:, :])

            for _rep in range(repeat):
                # MT[t, i] = sum_r G[r, t] * A.T[r, i]   -> (16, 512) in PSUM
                mt_ps = ps_mt.tile([RR, I], _f32, tag="mt_ps")
                nc.tensor.matmul(mt_ps, g_s, at_s, start=True, stop=True)

                if mode == "f32rk48":
                    mt3 = singles.tile([RR3, I], _f32r, tag="mt3")
                    mtl = singles.tile([RR, I], _f32r, tag="mtl")
                    # Mh = f32r(MT) -> rows 0-15
                    nc.scalar.copy(mt3[0:RR, :], mt_ps)
                    # Ml = f32r(MT - Mh) (base-0 scratch, engines cannot
                    # write at partition base 16)
                    nc.vector.tensor_sub(mtl, mt_ps, mt3[0:RR, :].bitcast(_f32))
                    # assemble rows 16-31 = Ml, rows 32-47 = Mh via DMA
                    nc.sync.dma_start(out=mt3[RR : 2 * RR, :], in_=mtl[:, :])
                    nc.sync.dma_start(out=mt3[2 * RR : RR3, :], in_=mt3[0:RR, :])
                else:
                    mt_s = singles.tile([RR, I], mm_dt, tag="mt")
                    nc.vector.tensor_copy(mt_s, mt_ps)

                for c in range(NCHUNK):
                    if mode == "f32rk48":
                        rh = rh_pool.tile([RR3, KCHUNK * J], _f32r, tag="rh")
                        # Rh = f32r(b*c) on all 48 rows, whole chunk in one
                        # DVE op via broadcast APs
                        kc = slice(c * KCHUNK, (c + 1) * KCHUNK)
                        nc.vector.tensor_mul(
                            rh[:, :].rearrange("p (k j) -> p k j", k=KCHUNK),
                            bt_s[:, :].unsqueeze(1).broadcast_to((RR3, KCHUNK, J)),
                            ct_s[:, kc].unsqueeze(2).broadcast_to((RR3, KCHUNK, J)),
                        )
                        # half-chunk granularity for the unrounded product
                        # (GPSIMD) and Rl = f32r(b*c - Rh) (DVE), rows 32-47
                        HC = KCHUNK // 2
                        for h in range(2):
                            khc = slice(c * KCHUNK + h * HC, c * KCHUNK + (h + 1) * HC)
                            jhc = slice(h * HC * J, (h + 1) * HC * J)
                            rf = rf_pool.tile([RR3, HC * J], _f32, tag="rf")
                            nc.gpsimd.tensor_mul(
                                rf[2 * RR : RR3, :].rearrange(
                                    "p (k j) -> p k j", k=HC
                                ),
                                bt_s[2 * RR : RR3, :]
                                .unsqueeze(1)
                                .broadcast_to((RR, HC, J)),
                                ct_s[2 * RR : RR3, khc]
                                .unsqueeze(2)
                                .broadcast_to((RR, HC, J)),
                            )
                            sub_eng = getattr(nc, sub_engine)
                            sub_eng.tensor_sub(
                                rh[2 * RR : RR3, jhc],
                                rf[2 * RR : RR3, :],
                                rh[2 * RR : RR3, jhc].bitcast(_f32),
                            )
                    else:
                        rh = rh_pool.tile([RR, KCHUNK * J], mm_dt, tag="rh")
                        for kl in range(KCHUNK):
                            k = c * KCHUNK + kl
                            js = slice(kl * J, (kl + 1) * J)
                            if kl % 2 == 0:
                                nc.scalar.mul(
                                    rh[:, js], bt_s[0:RR, :], ct_s[0:RR, k : k + 1]
                                )
                            else:
                                nc.vector.tensor_scalar_mul(
                                    rh[:, js], bt_s[0:RR, :], ct_s[0:RR, k : k + 1]
                                )

                    for m in range(MBLK):
                        ms = slice(m * 128, (m + 1) * 128)
                        ot = ot_pool.tile([128, KCHUNK * J], _f32, tag="ot")
                        for kl in range(KCHUNK):
                            js = slice(kl * J, (kl + 1) * J)
                            pt = ps.tile([128, J], _f32, tag="pt")
                            lhs = mt3 if mode == "f32rk48" else mt_s
                            nc.tensor.matmul(
                                pt, lhs[:, ms], rh[:, js], start=True, stop=True
                            )
                            # split the PSUM->SBUF copies ~1:3 DVE:ACT (DVE
                            # also carries the rhs build work)
                            if kl % copy_mod == 0:
                                nc.vector.tensor_copy(ot[:, js], pt)
                            else:
                                nc.scalar.copy(ot[:, js], pt)
                        nc.sync.dma_start(
                            out=o[ms, c * KCHUNK * J : (c + 1) * KCHUNK * J], in_=ot
                        )

    nc.finalize()
    return nc


_NC_CACHE: dict[str, object] = {}


def _host_prep(A, B, G):
    at = np.ascontiguousarray(A.T)                          # (4, 512)
    bt = np.ascontiguousarray(np.tile(B.T, (3 * R, 1)))     # (48, 512)
    g = np.ascontiguousarray(G)                             # (4, 16)
    return at, bt, g


def _ct_for_core(C, d):
    cs = C[d * KS : (d + 1) * KS]                           # (64, 4)
    return np.ascontiguousarray(np.tile(np.repeat(cs.T, R, axis=0), (3, 1)))  # (48, 64)


def kernel(A: np.ndarray, B: np.ndarray, C: np.ndarray, G: np.ndarray) -> np.ndarray:
    A = np.asarray(A, dtype=np.float32)
    B = np.asarray(B, dtype=np.float32)
    C = np.asarray(C, dtype=np.float32)
    G = np.asarray(G, dtype=np.float32)

    at, bt, g = _host_prep(A, B, G)

    if MATMUL_MODE not in _NC_CACHE:
        _NC_CACHE[MATMUL_MODE] = _build_nc(MATMUL_MODE)
    nc = _NC_CACHE[MATMUL_MODE]

    in_maps = [
        {"at": at, "g": g, "bt": bt, "ct": _ct_for_core(C, d)} for d in range(NCORES)
    ]
    res = run_bass_kernel_spmd(nc, in_maps, list(range(NCORES)))
    return np.concatenate([res.results[d]["o"] for d in range(NCORES)], axis=1)


# revision 14
# speedup vs baseline: 236.0604x; 4.1007x over previous
"""DirectTuckerNet forward on 8 Trainium2 NeuronCores.

    out = A @ G @ kron(C, B).T        # (I, K*J), fp32
    A: (512, 4)  B: (512, 4)  C: (512, 4)  G: (4, 16)

Equivalent per-element form (M = A @ G reshaped to (I, R3, R2)):
    out[i, k*J + j] = sum_{p,q} M[i, p*4+q] * C[k, p] * B[j, q]

Sharding: rows of C (the K dim) across the 8 cores.  Each core gets a
(KS=64)-row slice of C and produces the contiguous (512, KS*J) = 64 MiB
column block of the output, so the kernel is HBM-write bound.

The contraction (over the 16 (p,q) pairs) runs on the PE array.  Plain
fp32 matmul costs 4 cycles/row; the f32r (TF32-like, 12-bit mantissa)
path costs 1 cycle/row but loses ~1e-4.  Mode 'f32rk48' (default) gets
fp32-grade accuracy at f32r speed with a hi/lo split fused into ONE
K=48 matmul per output tile:

    x = xh + xl,  xh = f32r(x),  xl = f32r(x - xh)   (exact to ~2^-24)
    M @ R = Mh@Rh + Ml@Rh + Mh@Rl  (+ Ml@Rl ~ 2^-24, dropped)

  stationary MT3 (48, 512) = [Mh; Ml; Mh]     built once
  moving    RH3 (48, 512)  = [Rh; Rh; Rl]     per k:
      one 48-partition ACT scaled-copy writes Rh everywhere (f32r round),
      GPSIMD computes the unrounded product on rows 32-47,
      DVE overwrites rows 32-47 with Rl = f32r(b*c - Rh)
      (engine partition bases must be 32-aligned, hence this layout).

  psum (128,512) = MT3[:,m].T @ RH3[:,js]     one matmul per (m, k)
  PSUM -> SBUF copies alternate DVE/ACT; DMA out in 4 MiB chunks.

Host only does layout prep of the tiny operands (transpose/replicate,
no arithmetic) and the final concat of the 8 column blocks.

Modes: 'f32'     plain fp32 matmuls (slowest, exact)
       'f32r'    TF32-like single product (~1e-4 rel err)
       'f32rk48' fused hi/lo split, one K=48 matmul (fast, ~3e-7)
"""

import numpy as np

import concourse.bass as bass
import concourse.mybir as mybir
from concourse import bacc, tile
from concourse.bass_utils import run_bass_kernel_spmd

_f32 = mybir.dt.float32
_f32r = mybir.dt.float32r

I, J, K = 512, 512, 512
R = 4                      # R1 = R2 = R3
RR = R * R                 # 16, the contraction dim
RR3 = 3 * RR               # 48, the stacked contraction dim
NCORES = 8
KS = K // NCORES           # 64 k-rows per core
KCHUNK = 16                # k-values per staged output chunk
NCHUNK = KS // KCHUNK      # 4
MBLK = I // 128            # 4 row blocks of I

MATMUL_MODE = "f32rk48"


def _build_nc(
    mode: str = MATMUL_MODE,
    repeat: int = 1,
    *,
    sub_engine: str = "gpsimd",   # engine for the Rl subtraction
    ps_bufs: int = 7,
    ot_bufs: int = 2,
    copy_mod: int = 4,            # 1/copy_mod of PSUM copies go to DVE
):
    """repeat>1 re-runs the whole compute body (idempotent writes) so HW
    device time can be measured as a slope across repeat counts."""
    nc = bacc.Bacc()
    at = nc.dram_tensor("at", [R, I], _f32, kind="ExternalInput")       # A.T
    g = nc.dram_tensor("g", [R, RR], _f32, kind="ExternalInput")        # G
    # B.T replicated: row t of bt is B[:, t % 4] (3x16 rows).
    bt = nc.dram_tensor("bt", [RR3, J], _f32, kind="ExternalInput")
    # C-slice transposed+replicated: row t of ct is C[:, (t % 16) // 4].
    ct = nc.dram_tensor("ct", [RR3, KS], _f32, kind="ExternalInput")
    o = nc.dram_tensor("o", [I, KS * J], _f32, kind="ExternalOutput")

    mm_dt = _f32 if mode == "f32" else _f32r

    with tile.TileContext(nc) as tc:
        with (
            tc.tile_pool(name="singles", bufs=1) as singles,
            tc.tile_pool(name="ps_mt", bufs=1, space="PSUM") as ps_mt,
            tc.tile_pool(name="ps", bufs=ps_bufs, space="PSUM") as ps,
            tc.tile_pool(name="rh", bufs=2) as rh_pool,
            tc.tile_pool(name="rf", bufs=3) as rf_pool,
            tc.tile_pool(name="ot", bufs=ot_bufs) as ot_pool,
        ):
            at_s = singles.tile([R, I], _f32)
            g_s = singles.tile([R, RR], _f32)
            bt_s = singles.tile([RR3, J], _f32)
            ct_s = singles.tile([RR3, KS], _f32)
            nc.sync.dma_start(out=at_s, in_=at[:, :])
            nc.sync.dma_start(out=g_s, in_=g[:, :])
            nc.sync.dma_start(out=bt_s, in_=bt[:, :])
            nc.sync.dma_start(out=ct_s, in_=ct[:, :])

            for _rep in range(repeat):
                # MT[t, i] = sum_r G[r, t] * A.T[r, i]   -> (16, 512) in PSUM
                mt_ps = ps_mt.tile([RR, I], _f32, tag="mt_ps")
                nc.tensor.matmul(mt_ps, g_s, at_s, start=True, stop=True)

                if mode == "f32rk48":
                    mt3 = singles.tile([RR3, I], _f32r, tag="mt3")
                    mtl = singles.tile([RR, I], _f32r, tag="mtl")
                    # Mh = f32r(MT) -> rows 0-15
                    nc.scalar.copy(mt3[0:RR, :], mt_ps)
                    # Ml = f32r(MT - Mh) (base-0 scratch, engines cannot
                    # write at partition base 16)
                    nc.vector.tensor_sub(mtl, mt_ps, mt3[0:RR, :].bitcast(_f32))
                    # assemble rows 16-31 = Ml, rows 32-47 = Mh via DMA
                    nc.sync.dma_start(out=mt3[RR : 2 * RR, :], in_=mtl[:, :])
                    nc.sync.dma_start(out=mt3[2 * RR : RR3, :], in_=mt3[0:RR, :])
                else:
                    mt_s = singles.tile([RR, I], mm_dt, tag="mt")
                    nc.vector.tensor_copy(mt_s, mt_ps)

                for c in range(NCHUNK):
                    if mode == "f32rk48":
                        rh = rh_pool.tile([RR3, KCHUNK * J], _f32r, tag="rh")
                        # Rh = f32r(b*c) on all 48 rows, whole chunk in one
                        # DVE op via broadcast APs
                        kc = slice(c * KCHUNK, (c + 1) * KCHUNK)
                        nc.vector.tensor_mul(
                            rh[:, :].rearrange("p (k j) -> p k j", k=KCHUNK),
                            bt_s[:, :].unsqueeze(1).broadcast_to((RR3, KCHUNK, J)),
                            ct_s[:, kc].unsqueeze(2).broadcast_to((RR3, KCHUNK, J)),
                        )
                        # half-chunk granularity for the unrounded product
                        # (GPSIMD) and Rl = f32r(b*c - Rh) (DVE), rows 32-47
                        HC = KCHUNK // 2
                        for h in range(2):
                            khc = slice(c * KCHUNK + h * HC, c * KCHUNK + (h + 1) * HC)
                            jhc = slice(h * HC * J, (h + 1) * HC * J)
                            rf = rf_pool.tile([RR3, HC * J], _f32, tag="rf")
                            nc.gpsimd.tensor_mul(
                                rf[2 * RR : RR3, :].rearrange(
                                    "p (k j) -> p k j", k=HC
                                ),
                                bt_s[2 * RR : RR3, :]
                                .unsqueeze(1)
                                .broadcast_to((RR, HC, J)),
                                ct_s[2 * RR : RR3, khc]
                                .unsqueeze(2)
                                .broadcast_to((RR, HC, J)),
                            )
                            sub_eng = getattr(nc, sub_engine)
                            sub_eng.tensor_sub(
                                rh[2 * RR : RR3, jhc],
                                rf[2 * RR : RR3, :],
                                rh[2 * RR : RR3, jhc].bitcast(_f32),
                            )
                    else:
                        rh = rh_pool.tile([RR, KCHUNK * J], mm_dt, tag="rh")
                        for kl in range(KCHUNK):
                            k = c * KCHUNK + kl
                            js = slice(kl * J, (kl + 1) * J)
                            if kl % 2 == 0:
                                nc.scalar.mul(
                                    rh[:, js], bt_s[0:RR, :], ct_s[0:RR, k : k + 1]
                                )
                            else:
                                nc.vector.tensor_scalar_mul(
                                    rh[:, js], bt_s[0:RR, :], ct_s[0:RR, k : k + 1]
                                )

                    for m in range(MBLK):
                        ms = slice(m * 128, (m + 1) * 128)
                        ot = ot_pool.tile([128, KCHUNK * J], _f32, tag="ot")
                        for kl in range(KCHUNK):
                            js = slice(kl * J, (kl + 1) * J)
                            pt = ps.tile([128, J], _f32, tag="pt")
                            lhs = mt3 if mode == "f32rk48" else mt_s
                            nc.tensor.matmul(
                                pt, lhs[:, ms], rh[:, js], start=True, stop=True
                            )
                            # split the PSUM->SBUF copies ~1:3 DVE:ACT (DVE
                            # also carries the rhs build work)
                            if kl % copy_mod == 0:
                                nc.vector.tensor_copy(ot[:, js], pt)
                            else:
                                nc.scalar.copy(ot[:, js], pt)
                        nc.sync.dma_start(
                            out=o[ms, c * KCHUNK * J : (c + 1) * KCHUNK * J], in_=ot
                        )

    nc.finalize()
    return nc


_NC_CACHE: dict[str, object] = {}
_RUNNER_CACHE: dict[str, object] = {}


def _make_runner(nc, n_cores=NCORES):
    """Cached jit(shard_map(bass_exec)) runner so repeated kernel() calls
    reuse one compiled NEFF.  Every operand is a direct jit parameter (the
    neuronx_cc hook requires it); the output-init operands exist only for
    buffer aliasing and are never read by the NEFF (the kernel writes every
    element of the output), so constant zeros are fine."""
    import jax
    from jax.sharding import Mesh, NamedSharding, PartitionSpec as P

    try:
        from jax.experimental.shard_map import shard_map
    except ImportError:
        from jax import shard_map

    from concourse.bass2jax import (
        _bass_exec_p,
        install_neuronx_cc_hook,
        partition_id_tensor,
    )

    install_neuronx_cc_hook()
    partition_name = nc.partition_id_tensor.name if nc.partition_id_tensor else None
    in_names, out_names, out_avals = [], [], []
    for alloc in nc.m.functions[0].allocations:
        if not isinstance(alloc, mybir.MemoryLocationSet):
            continue
        name = alloc.memorylocations[0].name
        if alloc.kind == "ExternalInput":
            if name != partition_name:
                in_names.append(name)
        elif alloc.kind == "ExternalOutput":
            out_names.append(name)
            out_avals.append(
                jax.core.ShapedArray(tuple(alloc.tensor_shape), mybir.dt.np(alloc.dtype))
            )
    all_names = list(in_names) + list(out_names)
    if partition_name is not None:
        all_names.append(partition_name)

    def _body(*args):
        operands = list(args)
        if partition_name is not None:
            operands.append(partition_id_tensor())
        outs = _bass_exec_p.bind(
            *operands,
            out_avals=tuple(out_avals),
            in_names=tuple(all_names),
            out_names=tuple(out_names),
            lowering_input_output_aliases=(),
            sim_require_finite=True,
            sim_require_nnan=True,
            nc=nc,
        )
        return tuple(outs)

    devices = jax.devices()[:n_cores]
    mesh = Mesh(np.asarray(devices), ("core",))
    n_args = len(in_names) + len(out_names)
    fn = jax.jit(
        shard_map(
            _body,
            mesh=mesh,
            in_specs=(P("core"),) * n_args,
            out_specs=(P("core"),) * len(out_names),
            check_rep=False,
        )
    )
    sh = NamedSharding(mesh, P("core"))
    zeros = [
        jax.device_put(
            np.zeros((n_cores * a.shape[0], *a.shape[1:]), a.dtype), sh
        )
        for a in out_avals
    ]
    return fn, sh, in_names, out_names, out_avals, zeros


def _host_prep(A, B, G):
    at = np.ascontiguousarray(A.T)                          # (4, 512)
    bt = np.ascontiguousarray(np.tile(B.T, (3 * R, 1)))     # (48, 512)
    g = np.ascontiguousarray(G)                             # (4, 16)
    return at, bt, g


def _ct_for_core(C, d):
    cs = C[d * KS : (d + 1) * KS]                           # (64, 4)
    return np.ascontiguousarray(np.tile(np.repeat(cs.T, R, axis=0), (3, 1)))  # (48, 64)


def kernel(A: np.ndarray, B: np.ndarray, C: np.ndarray, G: np.ndarray) -> np.ndarray:
    A = np.asarray(A, dtype=np.float32)
    B = np.asarray(B, dtype=np.float32)
    C = np.asarray(C, dtype=np.float32)
    G = np.asarray(G, dtype=np.float32)

    at, bt, g = _host_prep(A, B, G)
    per_core = {
        "at": at,
        "g": g,
        "bt": bt,
        "ct": [_ct_for_core(C, d) for d in range(NCORES)],
    }

    if MATMUL_MODE not in _NC_CACHE:
        _NC_CACHE[MATMUL_MODE] = _build_nc(MATMUL_MODE)
    nc = _NC_CACHE[MATMUL_MODE]

    try:
        import jax

        if MATMUL_MODE not in _RUNNER_CACHE:
            _RUNNER_CACHE[MATMUL_MODE] = _make_runner(nc)
        fn, sh, in_names, out_names, out_avals, zeros = _RUNNER_CACHE[MATMUL_MODE]
        args = []
        for name in in_names:
            v = per_core[name]
            cat = (
                np.concatenate(v, axis=0)
                if isinstance(v, list)
                else np.concatenate([v] * NCORES, axis=0)
            )
            args.append(jax.device_put(cat, sh))
        outs = fn(*args, *zeros)
        full = np.asarray(outs[out_names.index("o")])  # (8*I, KS*J)
        return np.concatenate(
            [full[d * I : (d + 1) * I] for d in range(NCORES)], axis=1
        )
    except Exception:
        in_maps = [
            {
                "at": at,
                "g": g,
                "bt": bt,
                "ct": per_core["ct"][d],
            }
            for d in range(NCORES)
        ]
        res = run_bass_kernel_spmd(nc, in_maps, list(range(NCORES)))
        return np.concatenate([res.results[d]["o"] for d in range(NCORES)], axis=1)
